# revision 6
# baseline (speedup 1.0000x reference)
"""Trainium2 Bass kernel for BatchedCrossColumnAttentionCompressed.

Strategy (sharding_hint): shard leading N (column) axis across the 8 cores.
Each core: LN -> (folded) compress projections -> quantize -> AllReduce of the
small [TOK, 2R] compressed tensor -> decompress -> causal SDPA -> out proj.

Host-side algebraic folding (exact linear-map collapses):
  - LN affine (w,b) folded into projection weights (biases are zero for the
    actual inputs -> bias paths elided at build time).
  - k/v D->D projection collapsed into the D->R compression: w_kc = k_comp @ w_k_eff.
  - col_mask folded into w_kc/w_vc; 1/n_active folded into decompress weights.
  - 1/sqrt(HD) folded into q projection.
Softmax: scores are tiny (|s| << 1), so max-subtraction is skipped (exact same
math as reference up to fp rounding). exp computed on ACT with accum_out
producing the per-row sums; normalization is fused into the PE transpose of
attn via multiplication with diag(1/Z).
quant_ste round() implemented with the fp32 magic-constant RNE trick.
"""

import numpy as np
import ml_dtypes

N, B, T, D = 8, 4, 1024, 512
H = 4
HD = D // H           # 128
R = 64
R2 = 2 * R            # 128
EPS = 1e-5
TOK = B * T           # 4096
NTI = TOK // 128      # 32 token chunks
KD = D // 128         # 4 contraction chunks
NQ = T // 128         # 8 q-chunks per batch row
MAGIC = 12582912.0    # 1.5 * 2^23 -> round-to-nearest-even trick
NEG = -30000.0

_STATE = {}


def _build_program(with_kv_bias, with_q_bias):
    from concourse import bacc
    import concourse.bass as bass
    import concourse.tile as tile
    import concourse.mybir as mybir

    f32 = mybir.dt.float32
    bf16 = mybir.dt.bfloat16
    AF = mybir.ActivationFunctionType
    ALU = mybir.AluOpType
    AX = mybir.AxisListType

    nc = bacc.Bacc("TRN2", target_bir_lowering=False, debug=False, num_devices=N)

    x_d = nc.dram_tensor("x", [TOK, D], f32, kind="ExternalInput").ap()
    wkv_d = nc.dram_tensor("wkv", [D, R2], bf16, kind="ExternalInput").ap()
    wq_d = nc.dram_tensor("wq", [D, D], bf16, kind="ExternalInput").ap()
    wo_d = nc.dram_tensor("wo", [D, D], bf16, kind="ExternalInput").ap()
    kdec_d = nc.dram_tensor("kdec", [R, D], bf16, kind="ExternalInput").ap()
    vdec_d = nc.dram_tensor("vdec", [R, D], bf16, kind="ExternalInput").ap()
    ident_d = nc.dram_tensor("ident", [128, 128], bf16, kind="ExternalInput").ap()
    negm_d = nc.dram_tensor("negm", [128, 128], f32, kind="ExternalInput").ap()
    if with_kv_bias:
        bkv_d = nc.dram_tensor("bkv", [1, R2], bf16, kind="ExternalInput").ap()
    if with_q_bias:
        bq_d = nc.dram_tensor("bq", [1, D], bf16, kind="ExternalInput").ap()
    out_d = nc.dram_tensor("out", [TOK, D], f32, kind="ExternalOutput").ap()

    with tile.TileContext(nc) as tc:
        with (
            tc.tile_pool(name="consts", bufs=1) as consts,
            tc.tile_pool(name="big", bufs=1) as big,
            tc.tile_pool(name="work", bufs=3) as work,
            tc.tile_pool(name="small", bufs=4) as small,
            tc.tile_pool(name="ps", bufs=4, space="PSUM") as ps,
            tc.tile_pool(name="psbig", bufs=2, space="PSUM") as psbig,
            tc.tile_pool(name="dram", bufs=1, space="DRAM") as dpool,
        ):
            ar_in = dpool.tile([TOK, R2], bf16, name="ar_in")
            ar_out = dpool.tile([TOK, R2], bf16, name="ar_out",
                                addr_space="Shared")

            # ---- constants ----
            ident = consts.tile([128, 128], bf16, name="ident")
            nc.sync.dma_start(out=ident, in_=ident_d)
            negm = consts.tile([128, 128], f32, name="negm")
            nc.sync.dma_start(out=negm, in_=negm_d)
            wkv_s = []
            for kd in range(KD):
                wkvt = consts.tile([128, R2], bf16, name=f"wkv{kd}")
                nc.sync.dma_start(out=wkvt, in_=wkv_d[kd * 128:(kd + 1) * 128, :])
                wkv_s.append(wkvt)
            wq_s = []
            for kd in range(KD):
                wqt = consts.tile([128, D], bf16, name=f"wq{kd}")
                nc.sync.dma_start(out=wqt, in_=wq_d[kd * 128:(kd + 1) * 128, :])
                wq_s.append(wqt)
            wo_s = []
            for h in range(H):
                wot = consts.tile([128, D], bf16, name=f"wo{h}")
                nc.sync.dma_start(out=wot, in_=wo_d[h * 128:(h + 1) * 128, :])
                wo_s.append(wot)
            kdec_s = consts.tile([R, D], bf16, name="kdec_s")
            nc.sync.dma_start(out=kdec_s, in_=kdec_d)
            vdec_s = consts.tile([R, D], bf16, name="vdec_s")
            nc.sync.dma_start(out=vdec_s, in_=vdec_d)
            eps_t = consts.tile([128, 1], f32, name="eps_t")
            nc.vector.memset(eps_t, EPS)
            ones_row = consts.tile([1, 512], bf16, name="ones_row")
            nc.vector.memset(ones_row, 1.0)
            if with_kv_bias:
                bkv_s = consts.tile([1, R2], bf16, name="bkv_s")
                nc.sync.dma_start(out=bkv_s, in_=bkv_d)
            if with_q_bias:
                bq_s = consts.tile([1, D], bf16, name="bq_s")
                nc.sync.dma_start(out=bq_s, in_=bq_d)

            # ---- persistent big tensors ----
            nt = [big.tile([128, TOK], bf16, tag=f"sh{kd}", name=f"nt{kd}")
                  for kd in range(KD)]
            qT = [big.tile([128, TOK], bf16, name=f"qT{h}") for h in range(H)]
            kT = [big.tile([128, TOK], bf16, name=f"kT{h}") for h in range(H)]
            vN = big.tile([128, NTI, D], bf16, name="vN")
            kavgT = big.tile([R, TOK], bf16, name="kavgT")
            vavgT = big.tile([R, TOK], bf16, name="vavgT")

            # ================= Phase A: LN + transpose + compress + quant ====
            with nc.named_scope("A_ln_compress"):
                for ti in range(NTI):
                    tsl = slice(ti * 128, (ti + 1) * 128)
                    xt = work.tile([128, D], f32, name="xt")
                    nc.sync.dma_start(out=xt, in_=x_d[tsl, :])
                    stats = small.tile([128, 6], f32, name="stats")
                    nc.vector.bn_stats(out=stats, in_=xt)
                    mv = small.tile([128, 2], f32, name="mv")
                    nc.vector.bn_aggr(out=mv, in_=stats)
                    std = small.tile([128, 1], f32, name="std")
                    nc.scalar.activation(out=std, in_=mv[:, 1:2], func=AF.Sqrt,
                                         bias=eps_t, scale=1.0)
                    rstd = small.tile([128, 1], f32, name="rstd")
                    nc.vector.reciprocal(out=rstd, in_=std)
                    nrm = work.tile([128, D], bf16, name="nrm")
                    nc.vector.tensor_scalar(out=nrm, in0=xt, scalar1=mv[:, 0:1],
                                            scalar2=rstd, op0=ALU.subtract,
                                            op1=ALU.mult)
                    for kd in range(KD):
                        pst = ps.tile([128, 128], bf16, tag="ps", name="pst")
                        nc.tensor.transpose(pst, nrm[:, kd * 128:(kd + 1) * 128],
                                            ident)
                        nc.scalar.copy(out=nt[kd][:, tsl], in_=pst)
                    pskv = ps.tile([128, R2], f32, tag="ps", name="pskv")
                    for kd in range(KD):
                        nc.tensor.matmul(pskv, lhsT=nt[kd][:, tsl], rhs=wkv_s[kd],
                                         start=(kd == 0),
                                         stop=(kd == KD - 1 and not with_kv_bias))
                    if with_kv_bias:
                        nc.tensor.matmul(pskv, lhsT=bkv_s, rhs=ones_row[:, 0:128],
                                         start=False, stop=True)
                    absm = small.tile([128, 2], f32, name="absm")
                    nc.vector.tensor_reduce(out=absm[:, 0:1], in_=pskv[:, 0:R],
                                            axis=AX.X, op=ALU.max,
                                            apply_absolute_value=True)
                    nc.vector.tensor_reduce(out=absm[:, 1:2], in_=pskv[:, R:R2],
                                            axis=AX.X, op=ALU.max,
                                            apply_absolute_value=True)
                    nc.vector.tensor_scalar_max(out=absm, in0=absm, scalar1=1e-8)
                    inv_s = small.tile([128, 2], f32, name="inv_s")
                    nc.vector.tensor_scalar_mul(out=inv_s, in0=absm,
                                                scalar1=1.0 / 127.0)
                    sc = small.tile([128, 2], f32, name="sc")
                    nc.vector.reciprocal(out=sc, in_=inv_s)
                    arq = work.tile([128, R2], bf16, name="arq")
                    tmpq = work.tile([128, R], f32, name="tmpq")
                    for half in range(2):
                        sl = slice(half * R, (half + 1) * R)
                        nc.vector.tensor_scalar_mul(out=tmpq, in0=pskv[:, sl],
                                                    scalar1=sc[:, half:half + 1])
                        nc.vector.tensor_scalar(out=tmpq, in0=tmpq, scalar1=MAGIC,
                                                scalar2=MAGIC, op0=ALU.add,
                                                op1=ALU.subtract)
                        nc.vector.tensor_scalar_mul(out=arq[:, sl], in0=tmpq,
                                                    scalar1=inv_s[:, half:half + 1])
                    nc.sync.dma_start(out=ar_in[tsl, :], in_=arq)

            # ================= Phase B: AllReduce ===========================
            with nc.named_scope("B_allreduce"):
                nc.gpsimd.collective_compute(
                    "AllReduce",
                    ALU.add,
                    replica_groups=[list(range(N))],
                    ins=[ar_in.opt()],
                    outs=[ar_out.opt()],
                )

            # ================= Phase C: q^T projection (overlaps AR) ========
            with nc.named_scope("C_qproj"):
                for h in range(H):
                    for nch in range(TOK // 512):
                        csl = slice(nch * 512, (nch + 1) * 512)
                        psq = ps.tile([128, 512], f32, tag="ps", name="psq")
                        for kd in range(KD):
                            nc.tensor.matmul(
                                psq,
                                lhsT=wq_s[kd][:, h * HD:(h + 1) * HD],
                                rhs=nt[kd][:, csl],
                                start=(kd == 0),
                                stop=(kd == KD - 1 and not with_q_bias),
                            )
                        if with_q_bias:
                            nc.tensor.matmul(psq,
                                             lhsT=bq_s[:, h * HD:(h + 1) * HD],
                                             rhs=ones_row, start=False, stop=True)
                        nc.vector.tensor_copy(out=qT[h][:, csl], in_=psq)

            # ================= Phase D: decompress k^T and v ================
            with nc.named_scope("D_decompress"):
                nc.sync.dma_start_transpose(out=kavgT, in_=ar_out[:, 0:R])
                nc.sync.dma_start_transpose(out=vavgT, in_=ar_out[:, R:R2])
                for h in range(H):
                    for nch in range(TOK // 512):
                        csl = slice(nch * 512, (nch + 1) * 512)
                        psd = ps.tile([128, 512], f32, tag="ps", name="psd")
                        nc.tensor.matmul(psd, lhsT=kdec_s[:, h * HD:(h + 1) * HD],
                                         rhs=kavgT[:, csl], start=True, stop=True)
                        nc.scalar.copy(out=kT[h][:, csl], in_=psd)
                for ti in range(NTI):
                    psv = ps.tile([128, 512], f32, tag="ps", name="psv")
                    nc.tensor.matmul(psv,
                                     lhsT=vavgT[:, ti * 128:(ti + 1) * 128],
                                     rhs=vdec_s, start=True, stop=True)
                    nc.vector.tensor_copy(out=vN[:, ti, :], in_=psv)

            # ================= Phase E: causal SDPA =========================
            outT = [big.tile([128, TOK], bf16, tag=f"sh{kd}", name=f"outT{kd}")
                    for kd in range(KD)]
            with nc.named_scope("E_sdpa"):
                for b in range(B):
                    base = b * T
                    for h in range(H):
                        sums = small.tile([128, NQ], f32, name="sums")
                        recips = small.tile([128, NQ], f32, name="recips")
                        for qi in range(NQ):
                            kext = (qi + 1) * 128
                            qsl = slice(base + qi * 128, base + (qi + 1) * 128)
                            pss = psbig.tile([128, 1024], f32, tag="pss",
                                             name="pss")
                            for ks0 in range(0, kext, 512):
                                ke = min(ks0 + 512, kext)
                                nc.tensor.matmul(
                                    pss[:, ks0:ke], lhsT=qT[h][:, qsl],
                                    rhs=kT[h][:, base + ks0:base + ke],
                                    start=True, stop=True)
                            nc.vector.tensor_tensor(out=pss[:, qi * 128:kext],
                                                    in0=pss[:, qi * 128:kext],
                                                    in1=negm, op=ALU.add)
                            attn = work.tile([128, 1024], bf16, name="attn")
                            nc.scalar.activation(out=attn[:, :kext],
                                                 in_=pss[:, :kext], func=AF.Exp,
                                                 accum_out=sums[:, qi:qi + 1])
                            nc.vector.reciprocal(out=recips[:, qi:qi + 1],
                                                 in_=sums[:, qi:qi + 1])
                            diag = small.tile([128, 128], bf16, name="diag")
                            nc.vector.tensor_scalar_mul(
                                out=diag, in0=ident,
                                scalar1=recips[:, qi:qi + 1])
                            attnT = work.tile([128, 1024], bf16, name="attnT")
                            ncopy = 0
                            for kg0 in range(0, qi + 1, 4):
                                kg1 = min(kg0 + 4, qi + 1)
                                psdt = ps.tile([128, 512], f32, tag="ps",
                                               name="psdt")
                                for ki in range(kg0, kg1):
                                    o = (ki - kg0) * 128
                                    nc.tensor.matmul(
                                        psdt[:, o:o + 128],
                                        lhsT=attn[:, ki * 128:(ki + 1) * 128],
                                        rhs=diag, start=True, stop=True)
                                w = (kg1 - kg0) * 128
                                if ncopy % 2 == 0:
                                    nc.vector.tensor_copy(
                                        out=attnT[:, kg0 * 128:kg0 * 128 + w],
                                        in_=psdt[:, 0:w])
                                else:
                                    nc.scalar.copy(
                                        out=attnT[:, kg0 * 128:kg0 * 128 + w],
                                        in_=psdt[:, 0:w])
                                ncopy += 1
                            psa = ps.tile([128, 128], f32, tag="ps", name="psa")
                            for ki in range(qi + 1):
                                nc.tensor.matmul(
                                    psa,
                                    lhsT=vN[:, b * NQ + ki, h * HD:(h + 1) * HD],
                                    rhs=attnT[:, ki * 128:(ki + 1) * 128],
                                    start=(ki == 0), stop=(ki == qi))
                            nc.scalar.copy(out=outT[h][:, qsl], in_=psa)

            # ================= Phase F: out proj + residual =================
            with nc.named_scope("F_outproj"):
                for ti in range(NTI):
                    tsl = slice(ti * 128, (ti + 1) * 128)
                    pso = ps.tile([128, 512], f32, tag="ps", name="pso")
                    for h in range(H):
                        nc.tensor.matmul(pso, lhsT=outT[h][:, tsl], rhs=wo_s[h],
                                         start=(h == 0), stop=(h == H - 1))
                    xt2 = work.tile([128, D], f32, name="xt")
                    nc.sync.dma_start(out=xt2, in_=x_d[tsl, :])
                    of = work.tile([128, D], f32, name="of")
                    nc.vector.tensor_add(out=of, in0=pso, in1=xt2)
                    nc.sync.dma_start(out=out_d[tsl, :], in_=of)

    nc.compile()
    return nc


def _prepare(inputs):
    bf = ml_dtypes.bfloat16
    x = np.ascontiguousarray(np.asarray(inputs["col_states"], np.float32))
    mask_f = np.asarray(inputs["col_mask"]).astype(np.float32)
    n_active = max(float(mask_f.sum()), 1.0)

    lw_kv = np.asarray(inputs["ln_kv_w"], np.float32).reshape(N, D)
    lb_kv = np.asarray(inputs["ln_kv_b"], np.float32).reshape(N, D)
    lw_q = np.asarray(inputs["ln_q_w"], np.float32).reshape(N, D)
    lb_q = np.asarray(inputs["ln_q_b"], np.float32).reshape(N, D)
    w_k = np.asarray(inputs["w_k"], np.float32)
    w_v = np.asarray(inputs["w_v"], np.float32)
    w_q = np.asarray(inputs["w_q"], np.float32)
    w_o = np.asarray(inputs["w_o"], np.float32)
    k_comp = np.asarray(inputs["k_comp"], np.float32)
    v_comp = np.asarray(inputs["v_comp"], np.float32)
    k_dec = np.asarray(inputs["k_dec"], np.float32)
    v_dec = np.asarray(inputs["v_dec"], np.float32)

    w_k_eff = w_k * lw_kv[:, None, :]
    w_v_eff = w_v * lw_kv[:, None, :]
    bias_k = np.einsum("ni,noi->no", lb_kv, w_k)
    bias_v = np.einsum("ni,noi->no", lb_kv, w_v)

    w_kc = np.einsum("nro,noi->nri", k_comp, w_k_eff) * mask_f[:, None, None]
    w_vc = np.einsum("nro,noi->nri", v_comp, w_v_eff) * mask_f[:, None, None]
    b_kc = np.einsum("no,nro->nr", bias_k, k_comp) * mask_f[:, None]
    b_vc = np.einsum("no,nro->nr", bias_v, v_comp) * mask_f[:, None]

    sc = 1.0 / np.sqrt(np.float32(HD))
    w_q_eff = (w_q * lw_q[:, None, :]) * sc
    b_q = np.einsum("ni,noi->no", lb_q, w_q) * sc

    k_dec_eff = k_dec / n_active
    v_dec_eff = v_dec / n_active

    with_kv_bias = bool(np.any(b_kc != 0) or np.any(b_vc != 0))
    with_q_bias = bool(np.any(b_q != 0))

    ident = np.eye(128, dtype=bf)
    negm = np.where(np.tril(np.ones((128, 128), bool)),
                    np.float32(0.0), np.float32(NEG)).astype(np.float32)

    in_maps = []
    for n in range(N):
        m = {
            "x": x[n].reshape(TOK, D),
            "wkv": np.ascontiguousarray(
                np.concatenate([w_kc[n].T, w_vc[n].T], axis=1)).astype(bf),
            "wq": np.ascontiguousarray(w_q_eff[n].T).astype(bf),
            "wo": np.ascontiguousarray(w_o[n].T).astype(bf),
            "kdec": np.ascontiguousarray(k_dec_eff.T).astype(bf),
            "vdec": np.ascontiguousarray(v_dec_eff.T).astype(bf),
            "ident": ident,
            "negm": negm,
        }
        if with_kv_bias:
            m["bkv"] = np.concatenate([b_kc[n], b_vc[n]])[None, :].astype(bf)
        if with_q_bias:
            m["bq"] = b_q[n][None, :].astype(bf)
        in_maps.append(m)
    return in_maps, with_kv_bias, with_q_bias


def _run(inputs, trace=False):
    from concourse import bass_utils

    in_maps, with_kv_bias, with_q_bias = _prepare(inputs)
    key = (with_kv_bias, with_q_bias)
    if key not in _STATE:
        _STATE[key] = _build_program(with_kv_bias, with_q_bias)
    nc = _STATE[key]
    res = bass_utils.run_bass_kernel_spmd(
        nc, in_maps, core_ids=list(range(N)), trace=trace
    )
    outs = np.stack([np.asarray(res.results[c]["out"]) for c in range(N)])
    out = outs.reshape(N, B, T, D)
    mask_b = np.asarray(inputs["col_mask"]).reshape(N, 1, 1, 1)
    out = np.where(mask_b, out,
                   np.asarray(inputs["col_states"], np.float32))
    return out, res


def kernel(**inputs):
    out, _ = _run(inputs, trace=False)
    return out


# revision 7
# speedup vs baseline: 2.1345x; 2.1345x over previous
"""Trainium2 Bass kernel for BatchedCrossColumnAttentionCompressed.

Strategy (sharding_hint): shard leading N (column) axis across the 8 cores.
Each core: LN -> (folded) compress projections -> quantize -> AllReduce of the
small [TOK, 2R] compressed tensor -> decompress -> causal SDPA -> out proj.

Host-side algebraic folding (exact linear-map collapses):
  - LN affine (w,b) folded into projection weights (biases are zero for the
    actual inputs -> bias paths elided at build time).
  - k/v D->D projection collapsed into the D->R compression: w_kc = k_comp @ w_k_eff.
  - col_mask folded into w_kc/w_vc; 1/n_active folded into decompress weights.
  - 1/sqrt(HD) folded into q projection.
Softmax: scores are tiny (|s| << 1), so max-subtraction is skipped (exact same
math as reference up to fp rounding). exp computed on ACT with accum_out
producing the per-row sums; normalization is fused into the PE transpose of
attn via multiplication with diag(1/Z).
quant_ste round() implemented with the fp32 magic-constant RNE trick.
"""

import numpy as np
import ml_dtypes

N, B, T, D = 8, 4, 1024, 512
H = 4
HD = D // H           # 128
R = 64
R2 = 2 * R            # 128
EPS = 1e-5
TOK = B * T           # 4096
NTI = TOK // 128      # 32 token chunks
KD = D // 128         # 4 contraction chunks
NQ = T // 128         # 8 q-chunks per batch row
MAGIC = 12582912.0    # 1.5 * 2^23 -> round-to-nearest-even trick
NEG = -30000.0

_STATE = {}


def _build_program(with_kv_bias, with_q_bias):
    from concourse import bacc
    import concourse.bass as bass
    import concourse.tile as tile
    import concourse.mybir as mybir

    f32 = mybir.dt.float32
    bf16 = mybir.dt.bfloat16
    AF = mybir.ActivationFunctionType
    ALU = mybir.AluOpType
    AX = mybir.AxisListType

    nc = bacc.Bacc("TRN2", target_bir_lowering=False, debug=False, num_devices=N)

    x_d = nc.dram_tensor("x", [TOK, D], f32, kind="ExternalInput").ap()
    wkv_d = nc.dram_tensor("wkv", [D, R2], bf16, kind="ExternalInput").ap()
    wq_d = nc.dram_tensor("wq", [D, D], bf16, kind="ExternalInput").ap()
    wo_d = nc.dram_tensor("wo", [D, D], bf16, kind="ExternalInput").ap()
    kdec_d = nc.dram_tensor("kdec", [R, D], bf16, kind="ExternalInput").ap()
    vdec_d = nc.dram_tensor("vdec", [R, D], bf16, kind="ExternalInput").ap()
    ident_d = nc.dram_tensor("ident", [128, 128], bf16, kind="ExternalInput").ap()
    negm_d = nc.dram_tensor("negm", [128, 128], f32, kind="ExternalInput").ap()
    if with_kv_bias:
        bkv_d = nc.dram_tensor("bkv", [1, R2], bf16, kind="ExternalInput").ap()
    if with_q_bias:
        bq_d = nc.dram_tensor("bq", [1, D], bf16, kind="ExternalInput").ap()
    out_d = nc.dram_tensor("out", [TOK, D], f32, kind="ExternalOutput").ap()

    with tile.TileContext(nc) as tc:
        with (
            tc.tile_pool(name="consts", bufs=1) as consts,
            tc.tile_pool(name="big", bufs=1) as big,
            tc.tile_pool(name="work", bufs=3) as work,
            tc.tile_pool(name="small", bufs=4) as small,
            tc.tile_pool(name="ps", bufs=4, space="PSUM") as ps,
            tc.tile_pool(name="psbig", bufs=2, space="PSUM") as psbig,
            tc.tile_pool(name="dram", bufs=1, space="DRAM") as dpool,
        ):
            ar_in = dpool.tile([TOK, R2], bf16, name="ar_in")
            ar_out = dpool.tile([TOK, R2], bf16, name="ar_out",
                                addr_space="Shared")

            # ---- constants ----
            ident = consts.tile([128, 128], bf16, name="ident")
            nc.sync.dma_start(out=ident, in_=ident_d)
            negm = consts.tile([128, 128], f32, name="negm")
            nc.sync.dma_start(out=negm, in_=negm_d)
            wkv_s = []
            for kd in range(KD):
                wkvt = consts.tile([128, R2], bf16, name=f"wkv{kd}")
                nc.sync.dma_start(out=wkvt, in_=wkv_d[kd * 128:(kd + 1) * 128, :])
                wkv_s.append(wkvt)
            wq_s = []
            for kd in range(KD):
                wqt = consts.tile([128, D], bf16, name=f"wq{kd}")
                nc.sync.dma_start(out=wqt, in_=wq_d[kd * 128:(kd + 1) * 128, :])
                wq_s.append(wqt)
            wo_s = []
            for h in range(H):
                wot = consts.tile([128, D], bf16, name=f"wo{h}")
                nc.sync.dma_start(out=wot, in_=wo_d[h * 128:(h + 1) * 128, :])
                wo_s.append(wot)
            kdec_s = consts.tile([R, D], bf16, name="kdec_s")
            nc.sync.dma_start(out=kdec_s, in_=kdec_d)
            vdec_s = consts.tile([R, D], bf16, name="vdec_s")
            nc.sync.dma_start(out=vdec_s, in_=vdec_d)
            eps_t = consts.tile([128, 1], f32, name="eps_t")
            nc.vector.memset(eps_t, EPS)
            ones_row = consts.tile([1, 512], bf16, name="ones_row")
            nc.vector.memset(ones_row, 1.0)
            if with_kv_bias:
                bkv_s = consts.tile([1, R2], bf16, name="bkv_s")
                nc.sync.dma_start(out=bkv_s, in_=bkv_d)
            if with_q_bias:
                bq_s = consts.tile([1, D], bf16, name="bq_s")
                nc.sync.dma_start(out=bq_s, in_=bq_d)

            # ---- persistent big tensors ----
            nt = [big.tile([128, TOK], bf16, tag=f"sh{kd}", name=f"nt{kd}")
                  for kd in range(KD)]
            qT = [big.tile([128, TOK], bf16, name=f"qT{h}") for h in range(H)]
            kT = [big.tile([128, TOK], bf16, name=f"kT{h}") for h in range(H)]
            vN = big.tile([128, NTI, D], bf16, name="vN")
            kavgT = big.tile([R, TOK], bf16, name="kavgT")
            vavgT = big.tile([R, TOK], bf16, name="vavgT")

            # ================= Phase A: LN + transpose + compress + quant ====
            with nc.named_scope("A_ln_compress"):
                for ti in range(NTI):
                    tsl = slice(ti * 128, (ti + 1) * 128)
                    xt = work.tile([128, D], f32, name="xt")
                    nc.sync.dma_start(out=xt, in_=x_d[tsl, :])
                    stats = small.tile([128, 6], f32, name="stats")
                    nc.vector.bn_stats(out=stats, in_=xt)
                    mv = small.tile([128, 2], f32, name="mv")
                    nc.vector.bn_aggr(out=mv, in_=stats)
                    std = small.tile([128, 1], f32, name="std")
                    nc.scalar.activation(out=std, in_=mv[:, 1:2], func=AF.Sqrt,
                                         bias=eps_t, scale=1.0)
                    rstd = small.tile([128, 1], f32, name="rstd")
                    nc.vector.reciprocal(out=rstd, in_=std)
                    nrm = work.tile([128, D], bf16, name="nrm")
                    nc.vector.tensor_scalar(out=nrm, in0=xt, scalar1=mv[:, 0:1],
                                            scalar2=rstd, op0=ALU.subtract,
                                            op1=ALU.mult)
                    for kd in range(KD):
                        pst = ps.tile([128, 128], bf16, tag="ps", name="pst")
                        nc.tensor.transpose(pst, nrm[:, kd * 128:(kd + 1) * 128],
                                            ident)
                        nc.scalar.copy(out=nt[kd][:, tsl], in_=pst)
                    pskv = ps.tile([128, R2], f32, tag="ps", name="pskv")
                    for kd in range(KD):
                        nc.tensor.matmul(pskv, lhsT=nt[kd][:, tsl], rhs=wkv_s[kd],
                                         start=(kd == 0),
                                         stop=(kd == KD - 1 and not with_kv_bias))
                    if with_kv_bias:
                        nc.tensor.matmul(pskv, lhsT=bkv_s, rhs=ones_row[:, 0:128],
                                         start=False, stop=True)
                    absm = small.tile([128, 2], f32, name="absm")
                    nc.vector.tensor_reduce(out=absm[:, 0:1], in_=pskv[:, 0:R],
                                            axis=AX.X, op=ALU.max,
                                            apply_absolute_value=True)
                    nc.vector.tensor_reduce(out=absm[:, 1:2], in_=pskv[:, R:R2],
                                            axis=AX.X, op=ALU.max,
                                            apply_absolute_value=True)
                    nc.vector.tensor_scalar_max(out=absm, in0=absm, scalar1=1e-8)
                    inv_s = small.tile([128, 2], f32, name="inv_s")
                    nc.vector.tensor_scalar_mul(out=inv_s, in0=absm,
                                                scalar1=1.0 / 127.0)
                    sc = small.tile([128, 2], f32, name="sc")
                    nc.vector.reciprocal(out=sc, in_=inv_s)
                    arq = work.tile([128, R2], bf16, name="arq")
                    tmpq = work.tile([128, R], f32, name="tmpq")
                    for half in range(2):
                        sl = slice(half * R, (half + 1) * R)
                        nc.vector.tensor_scalar_mul(out=tmpq, in0=pskv[:, sl],
                                                    scalar1=sc[:, half:half + 1])
                        nc.vector.tensor_scalar(out=tmpq, in0=tmpq, scalar1=MAGIC,
                                                scalar2=MAGIC, op0=ALU.add,
                                                op1=ALU.subtract)
                        nc.vector.tensor_scalar_mul(out=arq[:, sl], in0=tmpq,
                                                    scalar1=inv_s[:, half:half + 1])
                    nc.sync.dma_start(out=ar_in[tsl, :], in_=arq)

            # ================= Phase B: AllReduce ===========================
            with nc.named_scope("B_allreduce"):
                nc.gpsimd.collective_compute(
                    "AllReduce",
                    ALU.add,
                    replica_groups=[list(range(N))],
                    ins=[ar_in.opt()],
                    outs=[ar_out.opt()],
                )

            # ================= Phase C: q^T projection (overlaps AR) ========
            with nc.named_scope("C_qproj"):
                for h in range(H):
                    for nch in range(TOK // 512):
                        csl = slice(nch * 512, (nch + 1) * 512)
                        psq = ps.tile([128, 512], f32, tag="ps", name="psq")
                        for kd in range(KD):
                            nc.tensor.matmul(
                                psq,
                                lhsT=wq_s[kd][:, h * HD:(h + 1) * HD],
                                rhs=nt[kd][:, csl],
                                start=(kd == 0),
                                stop=(kd == KD - 1 and not with_q_bias),
                            )
                        if with_q_bias:
                            nc.tensor.matmul(psq,
                                             lhsT=bq_s[:, h * HD:(h + 1) * HD],
                                             rhs=ones_row, start=False, stop=True)
                        nc.vector.tensor_copy(out=qT[h][:, csl], in_=psq)

            # ================= Phase D: decompress k^T and v ================
            with nc.named_scope("D_decompress"):
                for ti in range(NTI):
                    tsl = slice(ti * 128, (ti + 1) * 128)
                    avgN = work.tile([128, R2], bf16, name="avgN")
                    nc.sync.dma_start(out=avgN, in_=ar_out[tsl, :])
                    psK = ps.tile([R, 128], f32, tag="ps", name="psK")
                    nc.tensor.matmul(psK, lhsT=avgN[:, 0:R], rhs=ident,
                                     start=True, stop=True)
                    nc.vector.tensor_copy(out=kavgT[:, tsl], in_=psK)
                    psV = ps.tile([R, 128], f32, tag="ps", name="psV")
                    nc.tensor.matmul(psV, lhsT=avgN[:, R:R2], rhs=ident,
                                     start=True, stop=True)
                    nc.scalar.copy(out=vavgT[:, tsl], in_=psV)
                for h in range(H):
                    for nch in range(TOK // 512):
                        csl = slice(nch * 512, (nch + 1) * 512)
                        psd = ps.tile([128, 512], f32, tag="ps", name="psd")
                        nc.tensor.matmul(psd, lhsT=kdec_s[:, h * HD:(h + 1) * HD],
                                         rhs=kavgT[:, csl], start=True, stop=True)
                        nc.scalar.copy(out=kT[h][:, csl], in_=psd)
                for ti in range(NTI):
                    psv = ps.tile([128, 512], f32, tag="ps", name="psv")
                    nc.tensor.matmul(psv,
                                     lhsT=vavgT[:, ti * 128:(ti + 1) * 128],
                                     rhs=vdec_s, start=True, stop=True)
                    nc.vector.tensor_copy(out=vN[:, ti, :], in_=psv)

            # ================= Phase E: causal SDPA =========================
            outT = [big.tile([128, TOK], bf16, tag=f"sh{kd}", name=f"outT{kd}")
                    for kd in range(KD)]
            with nc.named_scope("E_sdpa"):
                for b in range(B):
                    base = b * T
                    for h in range(H):
                        sums = small.tile([128, NQ], f32, name="sums")
                        recips = small.tile([128, NQ], f32, name="recips")
                        for qi in range(NQ):
                            kext = (qi + 1) * 128
                            qsl = slice(base + qi * 128, base + (qi + 1) * 128)
                            pss = psbig.tile([128, 1024], f32, tag="pss",
                                             name="pss")
                            for ks0 in range(0, kext, 512):
                                ke = min(ks0 + 512, kext)
                                nc.tensor.matmul(
                                    pss[:, ks0:ke], lhsT=qT[h][:, qsl],
                                    rhs=kT[h][:, base + ks0:base + ke],
                                    start=True, stop=True)
                            nc.vector.tensor_tensor(out=pss[:, qi * 128:kext],
                                                    in0=pss[:, qi * 128:kext],
                                                    in1=negm, op=ALU.add)
                            attn = work.tile([128, 1024], bf16, name="attn")
                            nc.scalar.activation(out=attn[:, :kext],
                                                 in_=pss[:, :kext], func=AF.Exp,
                                                 accum_out=sums[:, qi:qi + 1])
                            nc.vector.reciprocal(out=recips[:, qi:qi + 1],
                                                 in_=sums[:, qi:qi + 1])
                            diag = small.tile([128, 128], bf16, name="diag")
                            nc.vector.tensor_scalar_mul(
                                out=diag, in0=ident,
                                scalar1=recips[:, qi:qi + 1])
                            attnT = work.tile([128, 1024], bf16, name="attnT")
                            ncopy = 0
                            for kg0 in range(0, qi + 1, 4):
                                kg1 = min(kg0 + 4, qi + 1)
                                psdt = ps.tile([128, 512], f32, tag="ps",
                                               name="psdt")
                                for ki in range(kg0, kg1):
                                    o = (ki - kg0) * 128
                                    nc.tensor.matmul(
                                        psdt[:, o:o + 128],
                                        lhsT=attn[:, ki * 128:(ki + 1) * 128],
                                        rhs=diag, start=True, stop=True)
                                w = (kg1 - kg0) * 128
                                if ncopy % 2 == 0:
                                    nc.vector.tensor_copy(
                                        out=attnT[:, kg0 * 128:kg0 * 128 + w],
                                        in_=psdt[:, 0:w])
                                else:
                                    nc.scalar.copy(
                                        out=attnT[:, kg0 * 128:kg0 * 128 + w],
                                        in_=psdt[:, 0:w])
                                ncopy += 1
                            psa = ps.tile([128, 128], f32, tag="ps", name="psa")
                            for ki in range(qi + 1):
                                nc.tensor.matmul(
                                    psa,
                                    lhsT=vN[:, b * NQ + ki, h * HD:(h + 1) * HD],
                                    rhs=attnT[:, ki * 128:(ki + 1) * 128],
                                    start=(ki == 0), stop=(ki == qi))
                            nc.scalar.copy(out=outT[h][:, qsl], in_=psa)

            # ================= Phase F: out proj + residual =================
            with nc.named_scope("F_outproj"):
                for ti in range(NTI):
                    tsl = slice(ti * 128, (ti + 1) * 128)
                    pso = ps.tile([128, 512], f32, tag="ps", name="pso")
                    for h in range(H):
                        nc.tensor.matmul(pso, lhsT=outT[h][:, tsl], rhs=wo_s[h],
                                         start=(h == 0), stop=(h == H - 1))
                    xt2 = work.tile([128, D], f32, name="xt")
                    nc.sync.dma_start(out=xt2, in_=x_d[tsl, :])
                    of = work.tile([128, D], f32, name="of")
                    nc.vector.tensor_add(out=of, in0=pso, in1=xt2)
                    nc.sync.dma_start(out=out_d[tsl, :], in_=of)

    nc.compile()
    return nc


def _prepare(inputs):
    bf = ml_dtypes.bfloat16
    x = np.ascontiguousarray(np.asarray(inputs["col_states"], np.float32))
    mask_f = np.asarray(inputs["col_mask"]).astype(np.float32)
    n_active = max(float(mask_f.sum()), 1.0)

    lw_kv = np.asarray(inputs["ln_kv_w"], np.float32).reshape(N, D)
    lb_kv = np.asarray(inputs["ln_kv_b"], np.float32).reshape(N, D)
    lw_q = np.asarray(inputs["ln_q_w"], np.float32).reshape(N, D)
    lb_q = np.asarray(inputs["ln_q_b"], np.float32).reshape(N, D)
    w_k = np.asarray(inputs["w_k"], np.float32)
    w_v = np.asarray(inputs["w_v"], np.float32)
    w_q = np.asarray(inputs["w_q"], np.float32)
    w_o = np.asarray(inputs["w_o"], np.float32)
    k_comp = np.asarray(inputs["k_comp"], np.float32)
    v_comp = np.asarray(inputs["v_comp"], np.float32)
    k_dec = np.asarray(inputs["k_dec"], np.float32)
    v_dec = np.asarray(inputs["v_dec"], np.float32)

    w_k_eff = w_k * lw_kv[:, None, :]
    w_v_eff = w_v * lw_kv[:, None, :]
    bias_k = np.einsum("ni,noi->no", lb_kv, w_k)
    bias_v = np.einsum("ni,noi->no", lb_kv, w_v)

    w_kc = np.einsum("nro,noi->nri", k_comp, w_k_eff) * mask_f[:, None, None]
    w_vc = np.einsum("nro,noi->nri", v_comp, w_v_eff) * mask_f[:, None, None]
    b_kc = np.einsum("no,nro->nr", bias_k, k_comp) * mask_f[:, None]
    b_vc = np.einsum("no,nro->nr", bias_v, v_comp) * mask_f[:, None]

    sc = 1.0 / np.sqrt(np.float32(HD))
    w_q_eff = (w_q * lw_q[:, None, :]) * sc
    b_q = np.einsum("ni,noi->no", lb_q, w_q) * sc

    k_dec_eff = k_dec / n_active
    v_dec_eff = v_dec / n_active

    with_kv_bias = bool(np.any(b_kc != 0) or np.any(b_vc != 0))
    with_q_bias = bool(np.any(b_q != 0))

    ident = np.eye(128, dtype=bf)
    negm = np.where(np.tril(np.ones((128, 128), bool)),
                    np.float32(0.0), np.float32(NEG)).astype(np.float32)

    in_maps = []
    for n in range(N):
        m = {
            "x": x[n].reshape(TOK, D),
            "wkv": np.ascontiguousarray(
                np.concatenate([w_kc[n].T, w_vc[n].T], axis=1)).astype(bf),
            "wq": np.ascontiguousarray(w_q_eff[n].T).astype(bf),
            "wo": np.ascontiguousarray(w_o[n].T).astype(bf),
            "kdec": np.ascontiguousarray(k_dec_eff.T).astype(bf),
            "vdec": np.ascontiguousarray(v_dec_eff.T).astype(bf),
            "ident": ident,
            "negm": negm,
        }
        if with_kv_bias:
            m["bkv"] = np.concatenate([b_kc[n], b_vc[n]])[None, :].astype(bf)
        if with_q_bias:
            m["bq"] = b_q[n][None, :].astype(bf)
        in_maps.append(m)
    return in_maps, with_kv_bias, with_q_bias


def _run(inputs, trace=False):
    from concourse import bass_utils

    in_maps, with_kv_bias, with_q_bias = _prepare(inputs)
    key = (with_kv_bias, with_q_bias)
    if key not in _STATE:
        _STATE[key] = _build_program(with_kv_bias, with_q_bias)
    nc = _STATE[key]
    res = bass_utils.run_bass_kernel_spmd(
        nc, in_maps, core_ids=list(range(N)), trace=trace
    )
    outs = np.stack([np.asarray(res.results[c]["out"]) for c in range(N)])
    out = outs.reshape(N, B, T, D)
    mask_b = np.asarray(inputs["col_mask"]).reshape(N, 1, 1, 1)
    out = np.where(mask_b, out,
                   np.asarray(inputs["col_states"], np.float32))
    return out, res


def kernel(**inputs):
    out, _ = _run(inputs, trace=False)
    return out


# revision 17
# speedup vs baseline: 2.4379x; 1.1421x over previous
"""Trainium2 Bass kernel for BatchedCrossColumnAttentionCompressed.

Strategy (sharding_hint): shard leading N (column) axis across the 8 cores.
Each core: LN -> (folded) compress projections -> quantize -> AllReduce of the
small [TOK, 2R] compressed tensor -> decompress -> causal SDPA -> out proj.

Host-side algebraic folding (exact linear-map collapses):
  - LN affine (w,b) folded into projection weights (biases are zero for the
    actual inputs -> bias paths elided at build time).
  - k/v D->D projection collapsed into the D->R compression: w_kc = k_comp @ w_k_eff.
  - col_mask folded into w_kc/w_vc; 1/n_active folded into decompress weights.
  - 1/sqrt(HD) folded into q projection.
Softmax: scores are tiny (|s| << 1), so max-subtraction is skipped (exact same
math as reference up to fp rounding). exp computed on ACT with accum_out
producing the per-row sums; normalization is fused into the PE transpose of
attn via multiplication with diag(1/Z).
quant_ste round() implemented with the fp32 magic-constant RNE trick.
"""

import numpy as np
import ml_dtypes

N, B, T, D = 8, 4, 1024, 512
H = 4
HD = D // H           # 128
R = 64
R2 = 2 * R            # 128
EPS = 1e-5
TOK = B * T           # 4096
NTI = TOK // 128      # 32 token chunks
KD = D // 128         # 4 contraction chunks
NQ = T // 128         # 8 q-chunks per batch row
MAGIC = 12582912.0    # 1.5 * 2^23 -> round-to-nearest-even trick
NEG = -30000.0

_STATE = {}


def _build_program(with_kv_bias, with_q_bias):
    from concourse import bacc
    import concourse.bass as bass
    import concourse.tile as tile
    import concourse.mybir as mybir

    f32 = mybir.dt.float32
    bf16 = mybir.dt.bfloat16
    AF = mybir.ActivationFunctionType
    ALU = mybir.AluOpType
    AX = mybir.AxisListType

    nc = bacc.Bacc("TRN2", target_bir_lowering=False, debug=False, num_devices=N)

    x_d = nc.dram_tensor("x", [TOK, D], f32, kind="ExternalInput").ap()
    wkv_d = nc.dram_tensor("wkv", [D, R2], bf16, kind="ExternalInput").ap()
    wq_d = nc.dram_tensor("wq", [D, D], bf16, kind="ExternalInput").ap()
    wo_d = nc.dram_tensor("wo", [D, D], bf16, kind="ExternalInput").ap()
    kdec_d = nc.dram_tensor("kdec", [R, D], bf16, kind="ExternalInput").ap()
    vdec_d = nc.dram_tensor("vdec", [R, D], bf16, kind="ExternalInput").ap()
    ident_d = nc.dram_tensor("ident", [128, 128], bf16, kind="ExternalInput").ap()
    negm_d = nc.dram_tensor("negm", [128, 128], f32, kind="ExternalInput").ap()
    if with_kv_bias:
        bkv_d = nc.dram_tensor("bkv", [1, R2], bf16, kind="ExternalInput").ap()
    if with_q_bias:
        bq_d = nc.dram_tensor("bq", [1, D], bf16, kind="ExternalInput").ap()
    out_d = nc.dram_tensor("out", [TOK, D], f32, kind="ExternalOutput").ap()

    with tile.TileContext(nc) as tc:
        with (
            tc.tile_pool(name="consts", bufs=1) as consts,
            tc.tile_pool(name="big", bufs=1) as big,
            tc.tile_pool(name="work", bufs=3) as work,
            tc.tile_pool(name="small", bufs=4) as small,
            tc.tile_pool(name="ps", bufs=4, space="PSUM") as ps,
            tc.tile_pool(name="psbig", bufs=2, space="PSUM") as psbig,
            tc.tile_pool(name="dram", bufs=1, space="DRAM") as dpool,
        ):
            HALF = TOK // 2
            ar_in = dpool.tile([TOK, R2], bf16, name="ar_in")
            ar_out_a = dpool.tile([HALF, R2], bf16, name="ar_out_a",
                                  addr_space="Shared")
            ar_out_b = dpool.tile([HALF, R2], bf16, name="ar_out_b",
                                  addr_space="Shared")

            # ---- constants ----
            ident = consts.tile([128, 128], bf16, name="ident")
            nc.sync.dma_start(out=ident, in_=ident_d)
            negm = consts.tile([128, 128], f32, name="negm")
            nc.sync.dma_start(out=negm, in_=negm_d)
            wkv_s = []
            for kd in range(KD):
                wkvt = consts.tile([128, R2], bf16, name=f"wkv{kd}")
                nc.sync.dma_start(out=wkvt, in_=wkv_d[kd * 128:(kd + 1) * 128, :])
                wkv_s.append(wkvt)
            wq_s = []
            for kd in range(KD):
                wqt = consts.tile([128, D], bf16, name=f"wq{kd}")
                nc.sync.dma_start(out=wqt, in_=wq_d[kd * 128:(kd + 1) * 128, :])
                wq_s.append(wqt)
            wo_s = []
            for h in range(H):
                wot = consts.tile([128, D], bf16, name=f"wo{h}")
                nc.sync.dma_start(out=wot, in_=wo_d[h * 128:(h + 1) * 128, :])
                wo_s.append(wot)
            kdec_s = consts.tile([R, D], bf16, name="kdec_s")
            nc.sync.dma_start(out=kdec_s, in_=kdec_d)
            vdec_s = consts.tile([R, D], bf16, name="vdec_s")
            nc.sync.dma_start(out=vdec_s, in_=vdec_d)
            eps_t = consts.tile([128, 1], f32, name="eps_t")
            nc.vector.memset(eps_t, EPS)
            ones_row = consts.tile([1, 512], bf16, name="ones_row")
            nc.vector.memset(ones_row, 1.0)
            if with_kv_bias:
                bkv_s = consts.tile([1, R2], bf16, name="bkv_s")
                nc.sync.dma_start(out=bkv_s, in_=bkv_d)
            if with_q_bias:
                bq_s = consts.tile([1, D], bf16, name="bq_s")
                nc.sync.dma_start(out=bq_s, in_=bq_d)

            # ---- persistent big tensors ----
            # nt and outT share SBUF (disjoint lifetimes) via the same tag
            nt = big.tile([128, KD, TOK], bf16, tag="shbig", name="nt")
            qT = [big.tile([128, TOK], bf16, name=f"qT{h}") for h in range(H)]
            kT = [big.tile([128, TOK], bf16, name=f"kT{h}") for h in range(H)]
            vN = big.tile([128, NTI, D], bf16, name="vN")
            kavgT = big.tile([R, TOK], bf16, name="kavgT")
            vavgT = big.tile([R, TOK], bf16, name="vavgT")

            # ================= Phase A: LN + transpose + compress + quant ====
            with nc.named_scope("A_ln_compress"):
                for ti in range(NTI):
                    tsl = slice(ti * 128, (ti + 1) * 128)
                    xt = work.tile([128, D], f32, name="xt")
                    nc.sync.dma_start(out=xt, in_=x_d[tsl, :])
                    stats = small.tile([128, 6], f32, name="stats")
                    nc.vector.bn_stats(out=stats, in_=xt)
                    mv = small.tile([128, 2], f32, name="mv")
                    nc.vector.bn_aggr(out=mv, in_=stats)
                    std = small.tile([128, 1], f32, name="std")
                    nc.scalar.activation(out=std, in_=mv[:, 1:2], func=AF.Sqrt,
                                         bias=eps_t, scale=1.0)
                    rstd = small.tile([128, 1], f32, name="rstd")
                    nc.vector.reciprocal(out=rstd, in_=std)
                    # nbias = -mean*rstd ; normed = x*rstd + nbias  (on ACT)
                    nbias = small.tile([128, 1], f32, name="nbias")
                    nc.vector.tensor_scalar(out=nbias, in0=mv[:, 0:1],
                                            scalar1=rstd, scalar2=-1.0,
                                            op0=ALU.mult, op1=ALU.mult)
                    nrm = work.tile([128, D], bf16, name="nrm")
                    nc.scalar.activation(out=nrm, in_=xt, func=AF.Identity,
                                         bias=nbias, scale=rstd)
                    pst = ps.tile([128, KD * 128], bf16, tag="ps", name="pst")
                    for kd in range(KD):
                        nc.tensor.transpose(pst[:, kd * 128:(kd + 1) * 128],
                                            nrm[:, kd * 128:(kd + 1) * 128],
                                            ident)
                    nc.vector.tensor_copy(
                        out=nt[:, :, tsl],
                        in_=pst.rearrange("p (g c) -> p g c", g=KD))
                    pskv = ps.tile([128, R2], f32, tag="ps", name="pskv")
                    for kd in range(KD):
                        nc.tensor.matmul(pskv, lhsT=nt[:, kd, tsl], rhs=wkv_s[kd],
                                         start=(kd == 0),
                                         stop=(kd == KD - 1 and not with_kv_bias))
                    if with_kv_bias:
                        nc.tensor.matmul(pskv, lhsT=bkv_s, rhs=ones_row[:, 0:128],
                                         start=False, stop=True)
                    absm = small.tile([128, 2], f32, name="absm")
                    nc.vector.tensor_reduce(
                        out=absm,
                        in_=pskv.rearrange("p (g r) -> p g r", g=2),
                        axis=AX.X, op=ALU.max, apply_absolute_value=True)
                    # inv_s = max(absm,1e-8)/127 ; sc = 1/inv_s ; mb = -MAGIC*inv_s
                    inv_s = small.tile([128, 2], f32, name="inv_s")
                    nc.vector.tensor_scalar(out=inv_s, in0=absm, scalar1=1e-8,
                                            scalar2=1.0 / 127.0, op0=ALU.max,
                                            op1=ALU.mult)
                    sc = small.tile([128, 2], f32, name="sc")
                    nc.vector.reciprocal(out=sc, in_=inv_s)
                    mb = small.tile([128, 2], f32, name="mb")
                    nc.vector.tensor_scalar_mul(out=mb, in0=inv_s, scalar1=-MAGIC)
                    arq = work.tile([128, R2], bf16, name="arq")
                    tmpq = work.tile([128, R2], f32, name="tmpq")
                    for half in range(2):
                        sl = slice(half * R, (half + 1) * R)
                        hh = slice(half, half + 1)
                        # y = x*sc + MAGIC  (rounds to int in fp32 mantissa)
                        nc.scalar.activation(out=tmpq[:, sl], in_=pskv[:, sl],
                                             func=AF.Copy, bias=MAGIC,
                                             scale=sc[:, hh])
                        # q = (y - MAGIC)*inv_s = y*inv_s + mb
                        nc.scalar.activation(out=arq[:, sl], in_=tmpq[:, sl],
                                             func=AF.Identity, bias=mb[:, hh],
                                             scale=inv_s[:, hh])
                    nc.sync.dma_start(out=ar_in[tsl, :], in_=arq)

            # ================= Phase B: AllReduce (split for earlier start) ==
            with nc.named_scope("B_allreduce"):
                nc.gpsimd.collective_compute(
                    "AllReduce",
                    ALU.add,
                    replica_groups=[list(range(N))],
                    ins=[ar_in[0:HALF, :].opt()],
                    outs=[ar_out_a.opt()],
                )
                nc.gpsimd.collective_compute(
                    "AllReduce",
                    ALU.add,
                    replica_groups=[list(range(N))],
                    ins=[ar_in[HALF:TOK, :].opt()],
                    outs=[ar_out_b.opt()],
                )

            # ================= Phase C: q^T projection (overlaps AR) ========
            with nc.named_scope("C_qproj"):
                for h in range(H):
                    for nch in range(TOK // 512):
                        csl = slice(nch * 512, (nch + 1) * 512)
                        psq = ps.tile([128, 512], f32, tag="ps", name="psq")
                        for kd in range(KD):
                            nc.tensor.matmul(
                                psq,
                                lhsT=wq_s[kd][:, h * HD:(h + 1) * HD],
                                rhs=nt[:, kd, csl],
                                start=(kd == 0),
                                stop=(kd == KD - 1 and not with_q_bias),
                            )
                        if with_q_bias:
                            nc.tensor.matmul(psq,
                                             lhsT=bq_s[:, h * HD:(h + 1) * HD],
                                             rhs=ones_row, start=False, stop=True)
                        nc.vector.tensor_copy(out=qT[h][:, csl], in_=psq)

            # ================= Phase D: decompress k^T and v ================
            with nc.named_scope("D_decompress"):
                for ti in range(NTI):
                    tsl = slice(ti * 128, (ti + 1) * 128)
                    avgN = work.tile([128, R2], bf16, name="avgN")
                    if ti < NTI // 2:
                        src = ar_out_a[ti * 128:(ti + 1) * 128, :]
                    else:
                        src = ar_out_b[(ti - NTI // 2) * 128:
                                       (ti - NTI // 2 + 1) * 128, :]
                    nc.sync.dma_start(out=avgN, in_=src)
                    psK = ps.tile([R, 128], f32, tag="ps", name="psK")
                    nc.tensor.matmul(psK, lhsT=avgN[:, 0:R], rhs=ident,
                                     start=True, stop=True)
                    nc.vector.tensor_copy(out=kavgT[:, tsl], in_=psK)
                    psV = ps.tile([R, 128], f32, tag="ps", name="psV")
                    nc.tensor.matmul(psV, lhsT=avgN[:, R:R2], rhs=ident,
                                     start=True, stop=True)
                    nc.scalar.copy(out=vavgT[:, tsl], in_=psV)
                for h in range(H):
                    for nch in range(TOK // 512):
                        csl = slice(nch * 512, (nch + 1) * 512)
                        psd = ps.tile([128, 512], f32, tag="ps", name="psd")
                        nc.tensor.matmul(psd, lhsT=kdec_s[:, h * HD:(h + 1) * HD],
                                         rhs=kavgT[:, csl], start=True, stop=True)
                        nc.scalar.copy(out=kT[h][:, csl], in_=psd)
                for ti in range(NTI):
                    psv = ps.tile([128, 512], f32, tag="ps", name="psv")
                    nc.tensor.matmul(psv,
                                     lhsT=vavgT[:, ti * 128:(ti + 1) * 128],
                                     rhs=vdec_s, start=True, stop=True)
                    nc.vector.tensor_copy(out=vN[:, ti, :], in_=psv)

            # ================= Phase E: causal SDPA =========================
            outT = big.tile([128, H, TOK], bf16, tag="shbig", name="outT")
            with nc.named_scope("E_sdpa"):
                for b in range(B):
                    base = b * T
                    for h in range(H):
                        sums = small.tile([128, NQ], f32, name="sums")
                        recips = small.tile([128, NQ], f32, name="recips")
                        for qi in range(NQ):
                            kext = (qi + 1) * 128
                            qsl = slice(base + qi * 128, base + (qi + 1) * 128)
                            pss = psbig.tile([128, 1024], f32, tag="pss",
                                             name="pss")
                            for ks0 in range(0, kext, 512):
                                ke = min(ks0 + 512, kext)
                                nc.tensor.matmul(
                                    pss[:, ks0:ke], lhsT=qT[h][:, qsl],
                                    rhs=kT[h][:, base + ks0:base + ke],
                                    start=True, stop=True)
                            nc.vector.tensor_tensor(out=pss[:, qi * 128:kext],
                                                    in0=pss[:, qi * 128:kext],
                                                    in1=negm, op=ALU.add)
                            attn = work.tile([128, 1024], bf16, name="attn")
                            nc.scalar.activation(out=attn[:, :kext],
                                                 in_=pss[:, :kext], func=AF.Exp,
                                                 accum_out=sums[:, qi:qi + 1])
                            nc.vector.reciprocal(out=recips[:, qi:qi + 1],
                                                 in_=sums[:, qi:qi + 1])
                            diag = small.tile([128, 128], bf16, name="diag")
                            nc.vector.tensor_scalar_mul(
                                out=diag, in0=ident,
                                scalar1=recips[:, qi:qi + 1])
                            attnT = work.tile([128, 1024], bf16, name="attnT")
                            ncopy = 0
                            for kg0 in range(0, qi + 1, 4):
                                kg1 = min(kg0 + 4, qi + 1)
                                psdt = ps.tile([128, 512], f32, tag="ps",
                                               name="psdt")
                                for ki in range(kg0, kg1):
                                    o = (ki - kg0) * 128
                                    nc.tensor.matmul(
                                        psdt[:, o:o + 128],
                                        lhsT=attn[:, ki * 128:(ki + 1) * 128],
                                        rhs=diag, start=True, stop=True)
                                w = (kg1 - kg0) * 128
                                nc.vector.tensor_copy(
                                    out=attnT[:, kg0 * 128:kg0 * 128 + w],
                                    in_=psdt[:, 0:w])
                                ncopy += 1
                            psa = ps.tile([128, 128], f32, tag="ps", name="psa")
                            for ki in range(qi + 1):
                                nc.tensor.matmul(
                                    psa,
                                    lhsT=vN[:, b * NQ + ki, h * HD:(h + 1) * HD],
                                    rhs=attnT[:, ki * 128:(ki + 1) * 128],
                                    start=(ki == 0), stop=(ki == qi))
                            nc.scalar.copy(out=outT[:, h, qsl], in_=psa)

            # ================= Phase F: out proj + residual =================
            with nc.named_scope("F_outproj"):
                for ti in range(NTI):
                    tsl = slice(ti * 128, (ti + 1) * 128)
                    pso = ps.tile([128, 512], f32, tag="ps", name="pso")
                    for h in range(H):
                        nc.tensor.matmul(pso, lhsT=outT[:, h, tsl], rhs=wo_s[h],
                                         start=(h == 0), stop=(h == H - 1))
                    xt2 = work.tile([128, D], f32, name="xt")
                    nc.sync.dma_start(out=xt2, in_=x_d[tsl, :])
                    of = work.tile([128, D], f32, name="of")
                    nc.vector.tensor_add(out=of, in0=pso, in1=xt2)
                    nc.sync.dma_start(out=out_d[tsl, :], in_=of)

    nc.compile()
    return nc


def _prepare(inputs):
    bf = ml_dtypes.bfloat16
    x = np.ascontiguousarray(np.asarray(inputs["col_states"], np.float32))
    mask_f = np.asarray(inputs["col_mask"]).astype(np.float32)
    n_active = max(float(mask_f.sum()), 1.0)

    lw_kv = np.asarray(inputs["ln_kv_w"], np.float32).reshape(N, D)
    lb_kv = np.asarray(inputs["ln_kv_b"], np.float32).reshape(N, D)
    lw_q = np.asarray(inputs["ln_q_w"], np.float32).reshape(N, D)
    lb_q = np.asarray(inputs["ln_q_b"], np.float32).reshape(N, D)
    w_k = np.asarray(inputs["w_k"], np.float32)
    w_v = np.asarray(inputs["w_v"], np.float32)
    w_q = np.asarray(inputs["w_q"], np.float32)
    w_o = np.asarray(inputs["w_o"], np.float32)
    k_comp = np.asarray(inputs["k_comp"], np.float32)
    v_comp = np.asarray(inputs["v_comp"], np.float32)
    k_dec = np.asarray(inputs["k_dec"], np.float32)
    v_dec = np.asarray(inputs["v_dec"], np.float32)

    w_k_eff = w_k * lw_kv[:, None, :]
    w_v_eff = w_v * lw_kv[:, None, :]
    bias_k = np.einsum("ni,noi->no", lb_kv, w_k)
    bias_v = np.einsum("ni,noi->no", lb_kv, w_v)

    w_kc = np.einsum("nro,noi->nri", k_comp, w_k_eff) * mask_f[:, None, None]
    w_vc = np.einsum("nro,noi->nri", v_comp, w_v_eff) * mask_f[:, None, None]
    b_kc = np.einsum("no,nro->nr", bias_k, k_comp) * mask_f[:, None]
    b_vc = np.einsum("no,nro->nr", bias_v, v_comp) * mask_f[:, None]

    sc = 1.0 / np.sqrt(np.float32(HD))
    w_q_eff = (w_q * lw_q[:, None, :]) * sc
    b_q = np.einsum("ni,noi->no", lb_q, w_q) * sc

    k_dec_eff = k_dec / n_active
    v_dec_eff = v_dec / n_active

    with_kv_bias = bool(np.any(b_kc != 0) or np.any(b_vc != 0))
    with_q_bias = bool(np.any(b_q != 0))

    ident = np.eye(128, dtype=bf)
    negm = np.where(np.tril(np.ones((128, 128), bool)),
                    np.float32(0.0), np.float32(NEG)).astype(np.float32)

    in_maps = []
    for n in range(N):
        m = {
            "x": x[n].reshape(TOK, D),
            "wkv": np.ascontiguousarray(
                np.concatenate([w_kc[n].T, w_vc[n].T], axis=1)).astype(bf),
            "wq": np.ascontiguousarray(w_q_eff[n].T).astype(bf),
            "wo": np.ascontiguousarray(w_o[n].T).astype(bf),
            "kdec": np.ascontiguousarray(k_dec_eff.T).astype(bf),
            "vdec": np.ascontiguousarray(v_dec_eff.T).astype(bf),
            "ident": ident,
            "negm": negm,
        }
        if with_kv_bias:
            m["bkv"] = np.concatenate([b_kc[n], b_vc[n]])[None, :].astype(bf)
        if with_q_bias:
            m["bq"] = b_q[n][None, :].astype(bf)
        in_maps.append(m)
    return in_maps, with_kv_bias, with_q_bias


def _run(inputs, trace=False):
    from concourse import bass_utils

    in_maps, with_kv_bias, with_q_bias = _prepare(inputs)
    key = (with_kv_bias, with_q_bias)
    if key not in _STATE:
        _STATE[key] = _build_program(with_kv_bias, with_q_bias)
    nc = _STATE[key]
    res = bass_utils.run_bass_kernel_spmd(
        nc, in_maps, core_ids=list(range(N)), trace=trace
    )
    outs = np.stack([np.asarray(res.results[c]["out"]) for c in range(N)])
    out = outs.reshape(N, B, T, D)
    mask_b = np.asarray(inputs["col_mask"]).reshape(N, 1, 1, 1)
    out = np.where(mask_b, out,
                   np.asarray(inputs["col_states"], np.float32))
    return out, res


def kernel(**inputs):
    out, _ = _run(inputs, trace=False)
    return out
